# revision 1
# baseline (speedup 1.0000x reference)
"""MetaPathEncoder (4x GraphConv + mean fusion) as a Bass/Tile SPMD kernel on 8 TRN2 cores.

Strategy (1D dst-node sharding, all 4 metapaths per core):
  - Each core owns 1250 output rows (10000/8). Edges are bucketed on host by
    (core, path, 128-row dst tile); both GraphConv norms and the 1/4 mean are
    folded into a single per-edge scale c_e computed on host.
  - On device, per (tile, path): dma_gather the edge source rows (bf16) from
    HBM, build the scaled one-hot matrix S[e, dst_local] = c_e on DVE
    (iota == dstl fused with * c_e), and segment-sum via PE matmuls
    accumulating in fp32 PSUM: h[dst, :] = sum_b S_b.T @ X_b.
  - h is transposed on the PE (identity matmul) to get fi-on-partitions, then
    16 accumulating matmuls apply the four 512x512 weights: out = sum_p h_p @ W_p.
  - Bias mean is added and the [1250, 512] fp32 result is DMA'd out; the host
    concatenates the 8 shards.
"""
import sys

for _p in ("/opt/trn_rl_repo",):
    if _p not in sys.path:
        sys.path.insert(0, _p)

import numpy as np
import ml_dtypes

import concourse.bass as bass
import concourse.tile as tile
from concourse import bacc, mybir
from concourse.bass_utils import run_bass_kernel_spmd
BF16 = ml_dtypes.bfloat16

N_NODES = 10000
N_PATHS = 4
IN_DIM = 512
OUT_DIM = 512
NCORES = 8
ROWS_PER_CORE = N_NODES // NCORES  # 1250
NTILES = (ROWS_PER_CORE + 127) // 128  # 10 (last tile has 98 rows)
NCALLS = NTILES * N_PATHS  # 40 gather calls per core

_program_cache: dict[int, object] = {}


def _build_program(B: int):
    """Build the SPMD Bass program for B gather blocks per (tile, path)."""
    if B in _program_cache:
        return _program_cache[B]

    dt = mybir.dt
    nc = bacc.Bacc("TRN2", target_bir_lowering=False, debug=False, num_devices=NCORES)

    featd = nc.dram_tensor("feat", [N_NODES, IN_DIM], dt.bfloat16, kind="ExternalInput").ap()
    idxd = nc.dram_tensor("idx", [128, NCALLS * B * 8], dt.int16, kind="ExternalInput").ap()
    dstld = nc.dram_tensor("dstl", [128, NCALLS * B], dt.float32, kind="ExternalInput").ap()
    ced = nc.dram_tensor("ce", [128, NCALLS * B], dt.float32, kind="ExternalInput").ap()
    wd = nc.dram_tensor("w", [128, 16 * OUT_DIM], dt.bfloat16, kind="ExternalInput").ap()
    bmd = nc.dram_tensor("bm", [128, OUT_DIM], dt.float32, kind="ExternalInput").ap()
    iotad = nc.dram_tensor("iota", [128, 128], dt.bfloat16, kind="ExternalInput").ap()
    identd = nc.dram_tensor("identity", [128, 128], dt.bfloat16, kind="ExternalInput").ap()
    outd = nc.dram_tensor("out", [ROWS_PER_CORE, OUT_DIM], dt.float32, kind="ExternalOutput").ap()

    with tile.TileContext(nc) as tc:
        with (
            tc.tile_pool(name="const", bufs=1) as cpool,
            tc.tile_pool(name="g", bufs=3) as gpool,
            tc.tile_pool(name="s", bufs=3) as spool,
            tc.tile_pool(name="hsb", bufs=3) as hsb_pool,
            tc.tile_pool(name="htsb", bufs=3) as htsb_pool,
            tc.tile_pool(name="osb", bufs=2) as osb_pool,
            tc.tile_pool(name="hps", bufs=2, space="PSUM") as hps_pool,
            tc.tile_pool(name="htps", bufs=2, space="PSUM") as htps_pool,
            tc.tile_pool(name="ops", bufs=2, space="PSUM") as ops_pool,
        ):
            idx_sb = cpool.tile([128, NCALLS * B * 8], dt.int16)
            nc.sync.dma_start(idx_sb[:], idxd[:])
            dstl_sb = cpool.tile([128, NCALLS * B], dt.float32)
            nc.sync.dma_start(dstl_sb[:], dstld[:])
            ce_sb = cpool.tile([128, NCALLS * B], dt.float32)
            nc.sync.dma_start(ce_sb[:], ced[:])
            w_sb = cpool.tile([128, 16 * OUT_DIM], dt.bfloat16)
            nc.sync.dma_start(w_sb[:], wd[:])
            bm_sb = cpool.tile([128, OUT_DIM], dt.float32)
            nc.sync.dma_start(bm_sb[:], bmd[:])
            iota_sb = cpool.tile([128, 128], dt.bfloat16)
            nc.sync.dma_start(iota_sb[:], iotad[:])
            ident = cpool.tile([128, 128], dt.bfloat16)
            nc.sync.dma_start(ident[:], identd[:])

            for t in range(NTILES):
                out_ps = ops_pool.tile([128, OUT_DIM], dt.float32)
                for p in range(N_PATHS):
                    call = t * N_PATHS + p
                    g = gpool.tile([128, B, IN_DIM], dt.bfloat16)
                    nc.gpsimd.dma_gather(
                        g[:],
                        featd[:],
                        idx_sb[:, call * B * 8 : (call + 1) * B * 8],
                        B * 128,
                        B * 128,
                        IN_DIM,
                        single_packet=False,
                    )
                    S = spool.tile([128, B * 128], dt.bfloat16)
                    for bb in range(B):
                        col = call * B + bb
                        nc.vector.tensor_scalar(
                            S[:, bb * 128 : (bb + 1) * 128],
                            iota_sb[:],
                            dstl_sb[:, col : col + 1],
                            ce_sb[:, col : col + 1],
                            op0=mybir.AluOpType.is_equal,
                            op1=mybir.AluOpType.mult,
                        )
                    hp = hps_pool.tile([128, IN_DIM], dt.float32)
                    for bb in range(B):
                        nc.tensor.matmul(
                            hp[:],
                            S[:, bb * 128 : (bb + 1) * 128],
                            g[:, bb, :],
                            start=(bb == 0),
                            stop=(bb == B - 1),
                        )
                    hs = hsb_pool.tile([128, IN_DIM], dt.bfloat16)
                    nc.scalar.copy(hs[:], hp[:])
                    htp = htps_pool.tile([128, IN_DIM], dt.bfloat16)
                    for cc in range(4):
                        nc.tensor.transpose(
                            htp[:, cc * 128 : (cc + 1) * 128],
                            hs[:, cc * 128 : (cc + 1) * 128],
                            ident[:],
                        )
                    hts = htsb_pool.tile([128, IN_DIM], dt.bfloat16)
                    nc.vector.tensor_copy(hts[:], htp[:])
                    for cc in range(4):
                        nc.tensor.matmul(
                            out_ps[:],
                            hts[:, cc * 128 : (cc + 1) * 128],
                            w_sb[:, (p * 4 + cc) * OUT_DIM : (p * 4 + cc + 1) * OUT_DIM],
                            start=(p == 0 and cc == 0),
                            stop=(p == N_PATHS - 1 and cc == 3),
                        )
                os_ = osb_pool.tile([128, OUT_DIM], dt.float32)
                nc.vector.tensor_add(os_[:], out_ps[:], bm_sb[:])
                rows = min(128, ROWS_PER_CORE - t * 128)
                nc.sync.dma_start(outd[t * 128 : t * 128 + rows, :], os_[:rows, :])

    nc.compile()
    _program_cache[B] = nc
    return nc


def _prep_host(feat, src, dst, W, b):
    """Host-side bucketing/padding. Returns (B, shared dict, per-core dicts)."""
    src = np.asarray(src).astype(np.int64)
    dst = np.asarray(dst).astype(np.int64)
    feat = np.asarray(feat, dtype=np.float32)
    W = np.asarray(W, dtype=np.float32)
    b = np.asarray(b, dtype=np.float32)

    feat_bf = feat.astype(BF16)

    # weights laid out [fi_local(128), p*4+chunk, fo] for direct SBUF residence
    Wt = np.empty((128, 16, OUT_DIM), dtype=BF16)
    for p in range(N_PATHS):
        for c in range(4):
            Wt[:, p * 4 + c, :] = W[p, c * 128 : (c + 1) * 128, :].astype(BF16)
    Wt = np.ascontiguousarray(Wt.reshape(128, 16 * OUT_DIM))

    bmean = b.mean(0).astype(np.float32)
    bm_bcast = np.ascontiguousarray(np.broadcast_to(bmean, (128, OUT_DIM)))

    iota_bf = np.ascontiguousarray(
        np.broadcast_to(np.arange(128, dtype=np.float32).astype(BF16), (128, 128))
    )

    # per-edge combined scale: deg_in(dst)^-1/2 * deg_out(src)^-1/2 * 1/4
    sorted_data = []  # per path: (src_sorted, dstl_unused, ce_sorted, dst_sorted)
    for p in range(N_PATHS):
        s, d = src[p], dst[p]
        deg_out = np.maximum(np.bincount(s, minlength=N_NODES), 1).astype(np.float64)
        deg_in = np.maximum(np.bincount(d, minlength=N_NODES), 1).astype(np.float64)
        ce = (deg_in[d] ** -0.5) * (deg_out[s] ** -0.5) * 0.25
        order = np.argsort(d, kind="stable")
        sorted_data.append((s[order], d[order], ce[order]))

    # (core, path, tile) ranges via searchsorted on per-path sorted dst
    bounds = []
    for c in range(NCORES):
        base = c * ROWS_PER_CORE
        for t in range(NTILES):
            lo = base + t * 128
            hi = base + min((t + 1) * 128, ROWS_PER_CORE)
            bounds.append((lo, hi))

    ranges = []  # [path][core*NTILES+t] = (a, b) into sorted arrays
    counts = np.zeros((N_PATHS, NCORES * NTILES), dtype=np.int64)
    for p in range(N_PATHS):
        ds = sorted_data[p][1]
        los = np.array([lo for lo, _ in bounds])
        his = np.array([hi for _, hi in bounds])
        a = np.searchsorted(ds, los, side="left")
        e = np.searchsorted(ds, his, side="left")
        ranges.append((a, e))
        counts[p] = e - a

    B = int(np.ceil(counts.max() / 128))

    per_core = []
    for c in range(NCORES):
        idxw = np.zeros((128, NCALLS * B * 8), dtype=np.int16)
        dstl_cols = np.full((128, NCALLS * B), 200.0, dtype=np.float32)
        ce_cols = np.zeros((128, NCALLS * B), dtype=np.float32)
        for t in range(NTILES):
            lo = c * ROWS_PER_CORE + t * 128
            for p in range(N_PATHS):
                call = t * N_PATHS + p
                a, e = ranges[p][0][c * NTILES + t], ranges[p][1][c * NTILES + t]
                cnt = e - a
                ss = sorted_data[p][0][a:e]
                dd = sorted_data[p][1][a:e] - lo
                cc = sorted_data[p][2][a:e]
                idx_pad = np.zeros(B * 128, dtype=np.int16)
                idx_pad[:cnt] = ss
                dstl_pad = np.full(B * 128, 200.0, dtype=np.float64)
                dstl_pad[:cnt] = dd
                ce_pad = np.zeros(B * 128, dtype=np.float64)
                ce_pad[:cnt] = cc
                # dma_gather wrapped index layout: position j -> [j%16, j//16],
                # replicated across the 8 groups of 16 partitions
                w16 = idx_pad.reshape(B * 8, 16).T  # [16, B*8]
                idxw[:, call * B * 8 : (call + 1) * B * 8] = np.tile(w16, (8, 1))
                dstl_cols[:, call * B : (call + 1) * B] = (
                    dstl_pad.reshape(B, 128).T.astype(np.float32)
                )
                ce_cols[:, call * B : (call + 1) * B] = (
                    ce_pad.reshape(B, 128).T.astype(np.float32)
                )
        per_core.append({"idx": idxw, "dstl": dstl_cols, "ce": ce_cols})

    shared = {
        "feat": feat_bf,
        "w": Wt,
        "bm": bm_bcast,
        "iota": iota_bf,
        "identity": np.eye(128, dtype=BF16),
    }
    return B, shared, per_core


def kernel(feat, src, dst, W, b):
    B, shared, per_core = _prep_host(feat, src, dst, W, b)
    nc = _build_program(B)
    in_maps = [{**shared, **pc} for pc in per_core]
    res = run_bass_kernel_spmd(nc, in_maps, list(range(NCORES)))
    out = np.concatenate([res.results[c]["out"] for c in range(NCORES)], axis=0)
    return out.astype(np.float32)


if __name__ == "__main__":
    rng = np.random.default_rng(0)
    feat = rng.standard_normal((N_NODES, IN_DIM), dtype=np.float32)
    src = rng.integers(0, N_NODES, (N_PATHS, 160000)).astype(np.int64)
    dst = rng.integers(0, N_NODES, (N_PATHS, 160000)).astype(np.int64)
    W = (rng.standard_normal((N_PATHS, IN_DIM, OUT_DIM), dtype=np.float32) / np.sqrt(IN_DIM)).astype(np.float32)
    b = np.zeros((N_PATHS, OUT_DIM), np.float32)
    out = kernel(feat=feat, src=src, dst=dst, W=W, b=b)
    print("kernel ran, out shape", out.shape, out.dtype)



# revision 2
# speedup vs baseline: 2.4197x; 2.4197x over previous
"""MetaPathEncoder (4x GraphConv + mean fusion) as a Bass/Tile SPMD kernel on 8 TRN2 cores.

Strategy (1D dst-node sharding, all 4 metapaths per core):
  - Each core owns 1250 output rows (10000/8). Edges are bucketed on host by
    (core, path, 128-row dst tile); both GraphConv norms and the 1/4 mean are
    folded into a per-edge scale c_e computed on host.
  - Per (tile, path) bucket the source indices are DEDUPLICATED on host; the
    scaled scatter matrix S[slot, dst_local] = sum of c_e over edges
    (uniq_src[slot] -> dst) is materialized densely on host (bf16) and DMA'd
    in, so no on-device one-hot construction (the DVE stays idle).
  - On device, per (tile, path): dma_gather the unique source rows (bf16)
    from HBM — gathers are issued round-robin on SWDGE queues 0-3 so the four
    Q7 core pairs can generate DMA descriptors concurrently — then
    segment-sum via PE matmuls accumulating in fp32 PSUM:
    h[dst, :] = sum_b S_b.T @ X_b.
  - h is transposed on the PE (identity matmul) to get fi-on-partitions, then
    16 accumulating matmuls apply the four 512x512 weights: out = sum_p h_p @ W_p.
  - Bias mean is added and the [1250, 512] fp32 result is DMA'd out; the host
    concatenates the 8 shards.
"""
import sys

for _p in ("/opt/trn_rl_repo",):
    if _p not in sys.path:
        sys.path.insert(0, _p)

import numpy as np
import ml_dtypes

import concourse.bass as bass
import concourse.tile as tile
from concourse import bacc, mybir
from concourse.bass_utils import run_bass_kernel_spmd
BF16 = ml_dtypes.bfloat16

N_NODES = 10000
N_PATHS = 4
IN_DIM = 512
OUT_DIM = 512
NCORES = 8
ROWS_PER_CORE = N_NODES // NCORES  # 1250
NTILES = (ROWS_PER_CORE + 127) // 128  # 10 (last tile has 98 rows)
NCALLS = NTILES * N_PATHS  # 40 gather calls per core

_program_cache: dict[tuple, object] = {}


def _build_program(Bc: tuple):
    """Build the SPMD Bass program; Bc[call] = gather blocks for call (t*4+p)."""
    if Bc in _program_cache:
        return _program_cache[Bc]

    TI = sum(Bc) * 8    # idx cols (int16, wrapped 16x, replicated 8x)
    TS = sum(Bc) * 128  # S cols (bf16)

    dt = mybir.dt
    nc = bacc.Bacc(
        "TRN2",
        target_bir_lowering=False,
        debug=False,
        num_devices=NCORES,
        num_swdge_queues=4,
    )

    featd = nc.dram_tensor("feat", [N_NODES, IN_DIM], dt.bfloat16, kind="ExternalInput").ap()
    idxd = nc.dram_tensor("idx", [128, TI], dt.int16, kind="ExternalInput").ap()
    sd = nc.dram_tensor("smat", [128, TS], dt.bfloat16, kind="ExternalInput").ap()
    wd = nc.dram_tensor("w", [128, 16 * OUT_DIM], dt.bfloat16, kind="ExternalInput").ap()
    bmd = nc.dram_tensor("bm", [128, OUT_DIM], dt.float32, kind="ExternalInput").ap()
    identd = nc.dram_tensor("identity", [128, 128], dt.bfloat16, kind="ExternalInput").ap()
    outd = nc.dram_tensor("out", [ROWS_PER_CORE, OUT_DIM], dt.float32, kind="ExternalOutput").ap()

    with tile.TileContext(nc) as tc:
        with (
            tc.tile_pool(name="const", bufs=1) as cpool,
            tc.tile_pool(name="g", bufs=5) as gpool,
            tc.tile_pool(name="s", bufs=4) as spool,
            tc.tile_pool(name="hsb", bufs=3) as hsb_pool,
            tc.tile_pool(name="htsb", bufs=3) as htsb_pool,
            tc.tile_pool(name="osb", bufs=2) as osb_pool,
            tc.tile_pool(name="hps", bufs=2, space="PSUM") as hps_pool,
            tc.tile_pool(name="htps", bufs=2, space="PSUM") as htps_pool,
            tc.tile_pool(name="ops", bufs=2, space="PSUM") as ops_pool,
        ):
            idx_sb = cpool.tile([128, TI], dt.int16)
            nc.sync.dma_start(idx_sb[:], idxd[:])
            w_sb = cpool.tile([128, 16 * OUT_DIM], dt.bfloat16)
            nc.sync.dma_start(w_sb[:], wd[:])
            bm_sb = cpool.tile([128, OUT_DIM], dt.float32)
            nc.sync.dma_start(bm_sb[:], bmd[:])
            ident = cpool.tile([128, 128], dt.bfloat16)
            nc.sync.dma_start(ident[:], identd[:])

            off_i = [0]
            off_s = [0]
            for b in Bc:
                off_i.append(off_i[-1] + b * 8)
                off_s.append(off_s[-1] + b * 128)

            for t in range(NTILES):
                out_ps = ops_pool.tile([128, OUT_DIM], dt.float32)
                for p in range(N_PATHS):
                    call = t * N_PATHS + p
                    B = Bc[call]
                    g = gpool.tile([128, B, IN_DIM], dt.bfloat16)
                    nc.gpsimd.dma_gather(
                        g[:],
                        featd[:],
                        idx_sb[:, off_i[call] : off_i[call + 1]],
                        B * 128,
                        B * 128,
                        IN_DIM,
                        single_packet=False,
                        queue_num=p,
                    )
                    S = spool.tile([128, B * 128], dt.bfloat16)
                    nc.sync.dma_start(S[:], sd[:, off_s[call] : off_s[call + 1]])
                    hp = hps_pool.tile([128, IN_DIM], dt.float32)
                    for bb in range(B):
                        nc.tensor.matmul(
                            hp[:],
                            S[:, bb * 128 : (bb + 1) * 128],
                            g[:, bb, :],
                            start=(bb == 0),
                            stop=(bb == B - 1),
                        )
                    hs = hsb_pool.tile([128, IN_DIM], dt.bfloat16)
                    nc.scalar.copy(hs[:], hp[:])
                    htp = htps_pool.tile([128, IN_DIM], dt.bfloat16)
                    for cc in range(4):
                        nc.tensor.transpose(
                            htp[:, cc * 128 : (cc + 1) * 128],
                            hs[:, cc * 128 : (cc + 1) * 128],
                            ident[:],
                        )
                    hts = htsb_pool.tile([128, IN_DIM], dt.bfloat16)
                    nc.vector.tensor_copy(hts[:], htp[:])
                    for cc in range(4):
                        nc.tensor.matmul(
                            out_ps[:],
                            hts[:, cc * 128 : (cc + 1) * 128],
                            w_sb[:, (p * 4 + cc) * OUT_DIM : (p * 4 + cc + 1) * OUT_DIM],
                            start=(p == 0 and cc == 0),
                            stop=(p == N_PATHS - 1 and cc == 3),
                        )
                os_ = osb_pool.tile([128, OUT_DIM], dt.float32)
                nc.vector.tensor_add(os_[:], out_ps[:], bm_sb[:])
                rows = min(128, ROWS_PER_CORE - t * 128)
                nc.sync.dma_start(outd[t * 128 : t * 128 + rows, :], os_[:rows, :])

    nc.compile()
    _program_cache[Bc] = nc
    return nc


def _prep_host(feat, src, dst, W, b):
    """Host-side bucketing, dedup, and S materialization.

    Returns (Bc tuple, shared dict, per-core dicts)."""
    src = np.asarray(src).astype(np.int64)
    dst = np.asarray(dst).astype(np.int64)
    feat = np.asarray(feat, dtype=np.float32)
    W = np.asarray(W, dtype=np.float32)
    b = np.asarray(b, dtype=np.float32)

    feat_bf = feat.astype(BF16)

    # weights laid out [fi_local(128), p*4+chunk, fo] for direct SBUF residence
    Wt = np.empty((128, 16, OUT_DIM), dtype=BF16)
    for p in range(N_PATHS):
        for c in range(4):
            Wt[:, p * 4 + c, :] = W[p, c * 128 : (c + 1) * 128, :].astype(BF16)
    Wt = np.ascontiguousarray(Wt.reshape(128, 16 * OUT_DIM))

    bmean = b.mean(0).astype(np.float32)
    bm_bcast = np.ascontiguousarray(np.broadcast_to(bmean, (128, OUT_DIM)))

    # per-edge combined scale: deg_in(dst)^-1/2 * deg_out(src)^-1/2 * 1/4
    sorted_data = []
    for p in range(N_PATHS):
        s, d = src[p], dst[p]
        deg_out = np.maximum(np.bincount(s, minlength=N_NODES), 1).astype(np.float64)
        deg_in = np.maximum(np.bincount(d, minlength=N_NODES), 1).astype(np.float64)
        ce = (deg_in[d] ** -0.5) * (deg_out[s] ** -0.5) * 0.25
        order = np.argsort(d, kind="stable")
        sorted_data.append((s[order], d[order], ce[order]))

    # (core, path, tile) buckets via searchsorted on per-path sorted dst
    bounds = []
    for c in range(NCORES):
        base = c * ROWS_PER_CORE
        for t in range(NTILES):
            lo = base + t * 128
            hi = base + min((t + 1) * 128, ROWS_PER_CORE)
            bounds.append((lo, hi))
    los = np.array([lo for lo, _ in bounds])
    his = np.array([hi for _, hi in bounds])

    ranges = []
    for p in range(N_PATHS):
        ds = sorted_data[p][1]
        a = np.searchsorted(ds, los, side="left")
        e = np.searchsorted(ds, his, side="left")
        ranges.append((a, e))

    # dedup per (core, path, tile); Bc[call] = max over cores
    buckets = {}  # (c, call) -> (uniq_idx, S_f32 [U, 128])
    Bc = np.zeros(NCALLS, dtype=np.int64)
    for c in range(NCORES):
        for t in range(NTILES):
            lo = c * ROWS_PER_CORE + t * 128
            for p in range(N_PATHS):
                call = t * N_PATHS + p
                a, e = ranges[p][0][c * NTILES + t], ranges[p][1][c * NTILES + t]
                ss = sorted_data[p][0][a:e]
                dl = (sorted_data[p][1][a:e] - lo).astype(np.int64)
                ce = sorted_data[p][2][a:e]
                uniq, inv = np.unique(ss, return_inverse=True)
                U = len(uniq)
                S = np.zeros((U, 128), dtype=np.float64)
                np.add.at(S, (inv, dl), ce)
                buckets[(c, call)] = (uniq, S)
                Bc[call] = max(Bc[call], (U + 127) // 128)
    Bc = np.maximum(Bc, 1)

    off_i = np.concatenate([[0], np.cumsum(Bc * 8)])
    off_s = np.concatenate([[0], np.cumsum(Bc * 128)])
    TI, TS = int(off_i[-1]), int(off_s[-1])

    per_core = []
    for c in range(NCORES):
        idxw = np.zeros((128, TI), dtype=np.int16)
        s_cols = np.zeros((128, TS), dtype=BF16)
        for call in range(NCALLS):
            B = int(Bc[call])
            uniq, S = buckets[(c, call)]
            U = len(uniq)
            idx_pad = np.zeros(B * 128, dtype=np.int16)
            idx_pad[:U] = uniq
            # dma_gather wrapped index layout: position j -> [j%16, j//16],
            # replicated across the 8 groups of 16 partitions
            w16 = idx_pad.reshape(B * 8, 16).T  # [16, B*8]
            idxw[:, off_i[call] : off_i[call + 1]] = np.tile(w16, (8, 1))
            S_pad = np.zeros((B * 128, 128), dtype=np.float32)
            S_pad[:U] = S
            s_cols[:, off_s[call] : off_s[call + 1]] = (
                S_pad.reshape(B, 128, 128).transpose(1, 0, 2).reshape(128, B * 128)
            ).astype(BF16)
        per_core.append({"idx": idxw, "smat": s_cols})

    shared = {
        "feat": feat_bf,
        "w": Wt,
        "bm": bm_bcast,
        "identity": np.eye(128, dtype=BF16),
    }
    return tuple(int(x) for x in Bc), shared, per_core


def kernel(feat, src, dst, W, b):
    Bc, shared, per_core = _prep_host(feat, src, dst, W, b)
    nc = _build_program(Bc)
    in_maps = [{**shared, **pc} for pc in per_core]
    res = run_bass_kernel_spmd(nc, in_maps, list(range(NCORES)))
    out = np.concatenate([res.results[c]["out"] for c in range(NCORES)], axis=0)
    return out.astype(np.float32)


if __name__ == "__main__":
    rng = np.random.default_rng(0)
    feat = rng.standard_normal((N_NODES, IN_DIM), dtype=np.float32)
    src = rng.integers(0, N_NODES, (N_PATHS, 160000)).astype(np.int64)
    dst = rng.integers(0, N_NODES, (N_PATHS, 160000)).astype(np.int64)
    W = (rng.standard_normal((N_PATHS, IN_DIM, OUT_DIM), dtype=np.float32) / np.sqrt(IN_DIM)).astype(np.float32)
    b = np.zeros((N_PATHS, OUT_DIM), np.float32)
    out = kernel(feat=feat, src=src, dst=dst, W=W, b=b)
    print("kernel ran, out shape", out.shape, out.dtype)
